# revision 15
# baseline (speedup 1.0000x reference)
"""DifferentiableWorld vocoder kernel for Trainium2 (8 NeuronCores, batch-parallel).

Bit-exact vs the XLA-CPU reference on the chaotic path (floor(cumsum(f0/SR))):
IEEE-exact interp + correctly-rounded /SR (Dekker) + XLA ReduceWindowRewriter
blocked scan (base 16) + exact floor. Spectral path (min-phase via DFT matmuls,
rfft/irfft, overlap-add, DC removal) at ordinary fp32 accuracy.
"""
import numpy as np
from contextlib import ExitStack

SR = 24000.0
FP = 640
B = 8
T = 4000
N = T * 256
SEG_P = 125
SEG_F = 8192
SL = 512              # pitch-path column slice
NSL = SEG_F // SL     # 16
T_TILE = 256
N_TILES = 16          # 15*256 + 160
f32 = np.float32


def _chunk_major(a, nchunks):
    rows = a.shape[0]
    pad = nchunks * 128 - rows
    if pad:
        a = np.concatenate([a, np.zeros((pad,) + a.shape[1:], a.dtype)], 0)
    return np.ascontiguousarray(
        a.reshape(nchunks, 128, -1).transpose(1, 0, 2).reshape(128, -1))


def _build_constants():
    c = {}
    r = np.arange(256)
    w = np.where(r < 128, (r + 128.5) / 256.0, (r - 127.5) / 256.0)
    c["W1"] = (1.0 - w).astype(f32)[None, :]
    c["W2"] = w.astype(f32)[None, :]

    r_hi = f32(np.float64(1.0) / SR)
    r_lo = f32(np.float64(1.0) / SR - np.float64(r_hi))
    rh1 = (np.array([r_hi]).view(np.int32) & np.int32(~0xFFF)).view(f32)[0]
    rh2 = f32(r_hi - rh1)
    c["div"] = (float(r_hi), float(r_lo), float(rh1), float(rh2))

    TWO_PI = 2 * np.pi
    c1 = f32(6.28125)
    c2 = f32(np.float64(TWO_PI) - np.float64(c1))
    c2 = (np.array([c2]).view(np.int32) & np.int32(~0xFFF)).view(f32)[0]
    c3 = f32(np.float64(TWO_PI) - np.float64(c1) - np.float64(c2))
    c["cw"] = (float(c1), float(c2), float(c3), float(1.0 / TWO_PI))

    n1 = np.arange(1, 513)
    j = np.arange(513)[:, None]
    nn = n1[None, :]
    Cc = np.where((j >= 1) & (j <= 511), 2.0 * np.cos(2 * np.pi * j * nn / 1024.0), 0.0)
    Cc[0, :] = 1.0
    Cc[512, :] = np.cos(np.pi * n1)
    Cc /= 1024.0
    scale = np.where(n1 <= 511, 2.0, 1.0)
    S2 = -np.sin(2 * np.pi * n1[:, None] * np.arange(513)[None, :] / 1024.0) * scale[:, None]
    A = 0.5 * (Cc @ S2)
    MA = np.zeros((FP, FP), dtype=f32)
    MA[:513, :513] = A.astype(f32)
    c["MA"] = _chunk_major(MA, 5)

    rr = np.arange(256)[:, None]
    kk = np.arange(513)[None, :]
    RC = np.zeros((256, FP), dtype=f32)
    RS = np.zeros((256, FP), dtype=f32)
    RC[:, :513] = np.cos(2 * np.pi * rr * kk / 1024.0).astype(f32)
    RS[:, :513] = (-np.sin(2 * np.pi * rr * kk / 1024.0)).astype(f32)
    c["RC"] = _chunk_major(RC, 2)
    c["RS"] = _chunk_major(RS, 2)

    ss = np.arange(512)[None, :]
    kk2 = np.arange(513)[:, None]
    gam = np.where((kk2 == 0) | (kk2 == 512), 1.0, 2.0)
    IC = np.zeros((FP, 512), dtype=f32)
    IS = np.zeros((FP, 512), dtype=f32)
    IC[:513, :] = (gam * np.cos(2 * np.pi * kk2 * ss / 1024.0) / 1024.0).astype(f32)
    IS[:513, :] = (-gam * np.sin(2 * np.pi * kk2 * ss / 1024.0) / 1024.0).astype(f32)
    c["IC"] = _chunk_major(IC, 5)
    c["IS"] = _chunk_major(IS, 5)

    v = np.arange(256)
    c["TRIS"] = _chunk_major((v[:, None] < v[None, :]).astype(f32), 2)
    c["ONESC"] = _chunk_major(np.ones((256, 1), dtype=f32), 2)
    c["EXP1"] = np.ones((1, 256), dtype=f32)
    c["IDN"] = np.eye(128, dtype=f32)
    return c


_CONST = _build_constants()


def _build_program():
    import concourse.bacc as bacc
    import concourse.bass as bass
    import concourse.tile as tile
    from concourse import mybir

    F32 = mybir.dt.float32
    I32 = mybir.dt.int32
    ALU = mybir.AluOpType
    ACT = mybir.ActivationFunctionType

    nc = bacc.Bacc("TRN2", target_bir_lowering=False, debug=False)

    def din(name, shape, dt=F32):
        return nc.dram_tensor(name, list(shape), dt, kind="ExternalInput").ap()

    g_d = din("g", (1, 4002))
    env_d = din("env", (513, T))
    apr_d = din("apr", (513, T))
    noi_d = din("noi", (1, N))
    W1_d = din("W1c", (1, 256))
    W2_d = din("W2c", (1, 256))
    MA_d = din("MAc", (128, 5 * FP))
    RC_d = din("RCc", (128, 2 * FP))
    RS_d = din("RSc", (128, 2 * FP))
    IC_d = din("ICc", (128, 5 * 512))
    IS_d = din("ISc", (128, 5 * 512))
    TRIS_d = din("TRISc", (128, 2 * 256))
    ONESC_d = din("ONESCc", (128, 2 * 1))
    EXP1_d = din("EXP1c", (1, 256))
    BND_d = din("BNDc", (1, 256))
    IDN_d = din("IDNc", (128, 128))

    out0_d = nc.dram_tensor("out0", [1, N], F32, kind="ExternalOutput").ap()
    out1_d = nc.dram_tensor("out1", [1, N], F32, kind="ExternalOutput").ap()

    r_hi, r_lo, rh1, rh2 = _CONST["div"]
    cw1, cw2, cw3, inv2pi = _CONST["cw"]
    HPI, PI, TPI = float(np.pi / 2), float(np.pi), float(2 * np.pi)

    with tile.TileContext(nc) as tc, ExitStack() as ctx:
        wpool = ctx.enter_context(tc.tile_pool(name="w", bufs=1))
        big = ctx.enter_context(tc.tile_pool(name="big", bufs=1))
        work = ctx.enter_context(tc.tile_pool(name="work", bufs=1))
        sm = ctx.enter_context(tc.tile_pool(name="sm", bufs=1))
        pp = ctx.enter_context(tc.tile_pool(name="pp", bufs=4, space="PSUM"))
        dpool = ctx.enter_context(tc.tile_pool(name="dram", bufs=1, space="DRAM"))

        def wload(ap_d, shape, tag):
            t = wpool.tile(list(shape), F32, tag=tag, name=tag)
            nc.sync.dma_start(out=t[:], in_=ap_d)
            return t

        MAt = wload(MA_d, (128, 5, FP), "MAt")
        RCt = wload(RC_d, (128, 2, FP), "RCt")
        RSt = wload(RS_d, (128, 2, FP), "RSt")
        ICt = wload(IC_d, (128, 5, 512), "ICt")
        ISt = wload(IS_d, (128, 5, 512), "ISt")
        TRISt = wload(TRIS_d, (128, 2, 256), "TRISt")
        ONESCt = wload(ONESC_d, (128, 2, 1), "ONESCt")
        EXP1t = wload(EXP1_d, (1, 256), "EXP1t")
        IDNt = wload(IDN_d, (128, 128), "IDNt")
        W1t = wpool.tile([SEG_P, 256], F32, tag="W1t", name="W1t")
        nc.sync.dma_start(out=W1t[:], in_=bass.AP(tensor=W1_d.tensor, offset=W1_d.offset,
                                                  ap=[[0, SEG_P], [1, 256]]))
        W2t = wpool.tile([SEG_P, 256], F32, tag="W2t", name="W2t")
        nc.sync.dma_start(out=W2t[:], in_=bass.AP(tensor=W2_d.tensor, offset=W2_d.offset,
                                                  ap=[[0, SEG_P], [1, 256]]))
        W1v = W1t[:].rearrange("p (h r) -> p h r", h=2)
        W2v = W2t[:].rearrange("p (h r) -> p h r", h=2)

        ft_dr = dpool.tile([SEG_P, SEG_F], F32, tag="ft_dr")
        X = big.tile([SEG_P, SEG_F], F32, tag="X", name="X")
        PM = big.tile([SEG_P, SEG_F], F32, tag="PM", name="PM")

        st = g_d.ap[1][0]
        JPS = SL // 256                    # frames (j) per slice = 2

        def p2k(tag, dt=F32):
            return work.tile([SEG_P, SL], dt, tag=tag, name=tag)

        # ---- interp + exact division, slice by slice; f0s spilled to DRAM
        for h in range(NSL):
            j0 = h * JPS
            gc = work.tile([SEG_P, JPS + 2], F32, tag="p_gc", name="gc")
            nc.sync.dma_start(out=gc[:], in_=bass.AP(tensor=g_d.tensor,
                                                     offset=g_d.offset + j0 * st,
                                                     ap=[[32 * st, SEG_P], [st, JPS + 2]]))
            fs = p2k("p_fs")
            fsv = fs[:].rearrange("p (j c r) -> p j c r", j=JPS, c=2)
            tmp_ = work.tile([SEG_P, 128], F32, tag="p_tmp", name="tmp_")
            for jj in range(JPS):
                # half 0 (r<128): g[t-1]*W1 + g[t]*W2 ; half 1: g[t]*W1 + g[t+1]*W2
                nc.vector.tensor_scalar(tmp_[:], W1v[:, 0, :], gc[:, jj:jj + 1],
                                        None, op0=ALU.mult)
                nc.vector.scalar_tensor_tensor(fsv[:, jj, 0, :], W2v[:, 0, :],
                                               gc[:, jj + 1:jj + 2], tmp_[:],
                                               op0=ALU.mult, op1=ALU.add)
                nc.vector.tensor_scalar(tmp_[:], W1v[:, 1, :], gc[:, jj + 1:jj + 2],
                                        None, op0=ALU.mult)
                nc.vector.scalar_tensor_tensor(fsv[:, jj, 1, :], W2v[:, 1, :],
                                               gc[:, jj + 2:jj + 3], tmp_[:],
                                               op0=ALU.mult, op1=ALU.add)
            if h == 0:
                nc.sync.dma_start(out=fs[0:1, 0:128], in_=BND_d[0:1, 0:128])
            if h == NSL - 1:
                nc.sync.dma_start(out=fs[SEG_P - 1:SEG_P, SL - 128:SL],
                                  in_=BND_d[0:1, 128:256])
            nc.sync.dma_start(out=ft_dr[:, h * SL:(h + 1) * SL], in_=fs[:])
            # Dekker division -> X slice
            xv = X[:, h * SL:(h + 1) * SL]
            t2 = p2k("p_t2"); pt = p2k("p_pt"); ac = p2k("p_ac"); tm = p2k("p_tm")
            nc.vector.tensor_scalar(xv, fs[:], 4097.0, None, op0=ALU.mult)
            nc.vector.tensor_sub(t2[:], xv, fs[:])
            nc.vector.tensor_sub(xv, xv, t2[:])                  # fh
            nc.vector.tensor_sub(t2[:], fs[:], xv)               # fl
            nc.vector.tensor_scalar(pt[:], fs[:], r_hi, None, op0=ALU.mult)
            nc.vector.tensor_scalar(ac[:], xv, rh1, None, op0=ALU.mult)
            nc.vector.tensor_sub(ac[:], ac[:], pt[:])
            nc.vector.tensor_scalar(tm[:], xv, rh2, None, op0=ALU.mult)
            nc.vector.tensor_add(ac[:], tm[:], ac[:])
            nc.vector.tensor_scalar(tm[:], t2[:], rh1, None, op0=ALU.mult)
            nc.vector.tensor_add(ac[:], tm[:], ac[:])
            nc.vector.tensor_scalar(tm[:], t2[:], rh2, None, op0=ALU.mult)
            nc.vector.tensor_add(ac[:], tm[:], ac[:])
            nc.vector.tensor_scalar(tm[:], fs[:], r_lo, None, op0=ALU.mult)
            nc.vector.tensor_add(ac[:], tm[:], ac[:])
            nc.vector.tensor_add(xv, pt[:], ac[:])               # q = f/SR

        # ---- blocked scan base 16 over X
        def inner16(view):
            for r in range(1, 16):
                nc.vector.tensor_add(view[:, :, r], view[:, :, r - 1], view[:, :, r])

        Xv = X[:].rearrange("p (c r) -> p c r", r=16)
        inner16(Xv)
        S1 = work.tile([SEG_P, 512], F32, tag="S1", name="S1")
        nc.vector.tensor_copy(S1[:], Xv[:, :, 15])
        S1v = S1[:].rearrange("p (c r) -> p c r", r=16)
        inner16(S1v)
        S2t = work.tile([SEG_P, 32], F32, tag="S2t", name="S2t")
        nc.vector.tensor_copy(S2t[:], S1v[:, :, 15])
        S2v = S2t[:].rearrange("p (c r) -> p c r", r=16)
        inner16(S2v)
        S3 = sm.tile([SEG_P, 2], F32, tag="S3", name="S3")
        nc.vector.tensor_copy(S3[:], S2v[:, :, 15])
        T3 = sm.tile([1, 256], F32, tag="T3", name="T3")
        nc.vector.memset(T3[:], 0.0)
        nc.sync.dma_start(out=T3[:, 0:250], in_=S3[:])
        T3v = T3[:].rearrange("p (c r) -> p c r", r=16)
        inner16(T3v)
        S4 = sm.tile([1, 16], F32, tag="S4", name="S4")
        nc.vector.tensor_copy(S4[:], T3v[:, :, 15])
        O4 = sm.tile([1, 16], F32, tag="O4", name="O4")
        Z16 = sm.tile([1, 16], F32, tag="Z16", name="Z16")
        nc.vector.memset(Z16[:], 0.0)
        nc.vector.tensor_tensor_scan(O4[:], S4[:], Z16[:], 0.0, ALU.add, ALU.bypass)
        O4x = sm.tile([1, 16], F32, tag="O4x", name="O4x")
        nc.vector.memset(O4x[:, 0:1], 0.0)
        nc.vector.tensor_copy(O4x[:, 1:16], O4[:, 0:15])
        for r in range(16):
            nc.vector.tensor_add(T3v[:, :, r], T3v[:, :, r], O4x[:])
        SH3s = sm.tile([1, 251], F32, tag="SH3s", name="SH3s")
        nc.vector.memset(SH3s[:, 0:1], 0.0)
        nc.vector.tensor_copy(SH3s[:, 1:251], T3[:, 0:250])
        SH3 = sm.tile([SEG_P, 2], F32, tag="SH3", name="SH3")
        nc.sync.dma_start(out=SH3[:], in_=SH3s[:, 0:250])
        for cc in range(2):
            nc.vector.tensor_scalar(S2v[:, cc, :], S2v[:, cc, :], SH3[:, cc:cc + 1],
                                    None, op0=ALU.add)
        SH2 = work.tile([SEG_P, 32], F32, tag="SH2", name="SH2")
        nc.vector.memset(SH2[:, 0:1], 0.0)
        nc.vector.tensor_copy(SH2[:, 1:32], S2t[:, 0:31])
        nc.sync.dma_start(out=SH2[1:SEG_P, 0:1], in_=S2t[0:SEG_P - 1, 31:32])
        for r in range(16):
            nc.vector.tensor_add(S1v[:, :, r], S1v[:, :, r], SH2[:])
        SH1 = work.tile([SEG_P, 512], F32, tag="SH1", name="SH1")
        nc.vector.memset(SH1[:, 0:1], 0.0)
        nc.vector.tensor_copy(SH1[:, 1:512], S1[:, 0:511])
        nc.sync.dma_start(out=SH1[1:SEG_P, 0:1], in_=S1[0:SEG_P - 1, 511:512])
        for r in range(16):
            nc.vector.tensor_add(Xv[:, :, r], Xv[:, :, r], SH1[:])

        # ---- exact floor + pitch + amp -> PM (pulse train), slice by slice
        pcar = sm.tile([SEG_P, 1], F32, tag="pcar", name="pcar")   # prev-slice last phase
        ph0 = sm.tile([SEG_P, 1], F32, tag="ph0", name="ph0")      # phase col 0 (f=0)
        for h in range(NSL):
            s_ = slice(h * SL, (h + 1) * SL)
            ph = p2k("p_ph")
            phi = p2k("p_phi", I32)
            tm = p2k("p_tm2")
            nc.vector.tensor_scalar(ph[:], X[:, s_], 0.5, None, op0=ALU.subtract)
            nc.vector.tensor_copy(phi[:], ph[:])
            nc.vector.tensor_copy(ph[:], phi[:])
            nc.vector.tensor_sub(tm[:], X[:, s_], ph[:])
            nc.vector.tensor_scalar(tm[:], tm[:], 1.0, None, op0=ALU.is_equal)
            nc.vector.tensor_add(ph[:], ph[:], tm[:])            # floor
            # pitch marks
            pmv = PM[:, s_]
            nc.vector.tensor_sub(pmv[:, 1:SL], ph[:, 1:SL], ph[:, 0:SL - 1])
            if h == 0:
                nc.vector.tensor_copy(ph0[:], ph[:, 0:1])
                nc.vector.tensor_copy(pmv[:, 0:1], ph[:, 0:1])   # placeholder, fixed later
            else:
                nc.vector.tensor_sub(pmv[:, 0:1], ph[:, 0:1], pcar[:])
            nc.vector.tensor_copy(pcar[:], ph[:, SL - 1:SL])
            # amp
            fr = p2k("p_fr")
            nc.sync.dma_start(out=fr[:], in_=ft_dr[:, s_])
            nc.vector.reciprocal(tm[:], fr[:])
            nc.vector.tensor_scalar(tm[:], tm[:], SR, 1.0, op0=ALU.mult, op1=ALU.max)
            nc.scalar.activation(tm[:], tm[:], ACT.Sqrt)
            # amp applied after col-0 fix for h==0, so stash amp col 0
            if h == 0:
                amp0 = sm.tile([SEG_P, 1], F32, tag="amp0", name="amp0")
                nc.vector.tensor_copy(amp0[:], tm[:, 0:1])
            nc.vector.tensor_mul(pmv, pmv, tm[:])
        # fix first column of each partition: pitch[p, 0] = ph0[p] - phase[p-1, 8191]
        pcs = sm.tile([SEG_P, 1], F32, tag="pcs", name="pcs")
        nc.vector.memset(pcs[0:1, :], 0.0)
        nc.sync.dma_start(out=pcs[1:SEG_P, :], in_=pcar[0:SEG_P - 1, :])
        nc.vector.tensor_sub(pcs[:], ph0[:], pcs[:])
        nc.vector.tensor_mul(pcs[:], pcs[:], amp0[:])
        nc.vector.tensor_copy(PM[:, 0:1], pcs[:])

        # ================= frame pipeline =================
        wp_dr = dpool.tile([256, T], F32, tag="wp_dr")
        wa_dr = dpool.tile([256, T], F32, tag="wa_dr")
        nst = noi_d.ap[1][0]

        carry = {s: [big.tile([128, 1], F32, tag=f"c{s}{h}", name=f"c{s}{h}")
                     for h in range(2)] for s in ("p", "a")}
        for s in ("p", "a"):
            for h in range(2):
                nc.vector.memset(carry[s][h][:], 0.0)

        for it in range(N_TILES):
            t0 = it * T_TILE
            tt = min(T_TILE, T - t0)
            nchunks = (tt + 127) // 128

            PF = [work.tile([128, T_TILE], F32, tag=f"PF{h}", name=f"PF{h}")
                  for h in range(2)]
            NF = [work.tile([128, T_TILE], F32, tag=f"NF{h}", name=f"NF{h}")
                  for h in range(2)]
            for ch in range(nchunks):
                tw = min(128, tt - ch * 128)
                stg = work.tile([128, 256], F32, tag="stg", name="stg")
                p0 = (t0 + ch * 128) // 32
                src = PM[p0:p0 + (tw + 31) // 32, :].rearrange("p (j r) -> p j r", r=256)
                nc.sync.dma_start(out=stg[:tw, :], in_=src)
                stgn = work.tile([128, 256], F32, tag="stgn", name="stgn")
                nc.sync.dma_start(out=stgn[:tw, :],
                                  in_=bass.AP(tensor=noi_d.tensor,
                                              offset=noi_d.offset + (t0 + ch * 128) * 256 * nst,
                                              ap=[[256 * nst, tw], [nst, 256]]))
                for hh in range(2):
                    ps = pp.tile([128, T_TILE], F32, tag="ps", name="ps")
                    nc.tensor.transpose(ps[:, :tw], stg[:tw, hh * 128:(hh + 1) * 128],
                                        IDNt[:tw, :tw])
                    nc.vector.tensor_copy(PF[hh][:, ch * 128:ch * 128 + tw], ps[:, :tw])
                    ps2 = pp.tile([128, T_TILE], F32, tag="ps", name="ps2")
                    nc.tensor.transpose(ps2[:, :tw], stgn[:tw, hh * 128:(hh + 1) * 128],
                                        IDNt[:tw, :tw])
                    nc.vector.tensor_copy(NF[hh][:, ch * 128:ch * 128 + tw], ps2[:, :tw])

            def rfft(frames, tag):
                re, im = [], []
                for m in range(5):
                    for lst, Wm, pfx in ((re, RCt, "r"), (im, RSt, "i")):
                        ps = pp.tile([128, T_TILE], F32, tag="ps", name="psf")
                        for hh in range(2):
                            nc.tensor.matmul(ps[:, :tt],
                                             Wm[:, hh, m * 128:(m + 1) * 128],
                                             frames[hh][:, :tt],
                                             start=(hh == 0), stop=(hh == 1))
                        xt_ = work.tile([128, T_TILE], F32,
                                        tag=f"X{tag}{pfx}{m}", name="xt_")
                        nc.vector.tensor_copy(xt_[:, :tt], ps[:, :tt])
                        lst.append(xt_)
                return re, im

            XPre, XPim = rfft(PF, "p")
            XAre, XAim = rfft(NF, "a")

            LNP = [work.tile([128, T_TILE], F32, tag=f"LNP{m}", name=f"LNP{m}")
                   for m in range(5)]
            LNA = [work.tile([128, T_TILE], F32, tag=f"LNA{m}", name=f"LNA{m}")
                   for m in range(5)]
            PEt = [work.tile([128, T_TILE], F32, tag=f"PEt{m}", name=f"PEt{m}")
                   for m in range(5)]
            AEt = [work.tile([128, T_TILE], F32, tag=f"AEt{m}", name=f"AEt{m}")
                   for m in range(5)]
            for m in range(5):
                rows = 128 if m < 4 else 1
                ev = work.tile([128, T_TILE], F32, tag="stg", name="ev")
                av = work.tile([128, T_TILE], F32, tag="stgn", name="av")
                if m == 4:
                    nc.vector.memset(ev[:], 1.0)
                    nc.vector.memset(av[:], 0.5)
                nc.sync.dma_start(out=ev[:rows, :tt],
                                  in_=env_d[m * 128:m * 128 + rows, t0:t0 + tt])
                nc.sync.dma_start(out=av[:rows, :tt],
                                  in_=apr_d[m * 128:m * 128 + rows, t0:t0 + tt])
                nc.vector.tensor_mul(av[:, :tt], av[:, :tt], av[:, :tt])
                nc.vector.tensor_mul(av[:, :tt], ev[:, :tt], av[:, :tt])
                nc.vector.tensor_sub(ev[:, :tt], ev[:, :tt], av[:, :tt])
                nc.vector.tensor_scalar(ev[:, :tt], ev[:, :tt], 1e-20, None, op0=ALU.max)
                nc.vector.tensor_scalar(av[:, :tt], av[:, :tt], 1e-20, None, op0=ALU.max)
                nc.scalar.activation(PEt[m][:, :tt], ev[:, :tt], ACT.Sqrt)
                nc.scalar.activation(AEt[m][:, :tt], av[:, :tt], ACT.Sqrt)
                nc.scalar.activation(LNP[m][:, :tt], ev[:, :tt], ACT.Ln)
                nc.scalar.activation(LNA[m][:, :tt], av[:, :tt], ACT.Ln)

            def spec_mul(LN, ENV, Xre, Xim):
                for m in range(5):
                    ps = pp.tile([128, T_TILE], F32, tag="ps", name="psm")
                    for kc in range(5):
                        nc.tensor.matmul(ps[:, :tt],
                                         MAt[:, kc, m * 128:(m + 1) * 128],
                                         LN[kc][:, :tt],
                                         start=(kc == 0), stop=(kc == 4))
                    mp = work.tile([128, T_TILE], F32, tag="mp", name="mp")
                    nc.vector.tensor_copy(mp[:, :tt], ps[:, :tt])
                    kq = work.tile([128, T_TILE], F32, tag="kq", name="kq")
                    ki = work.tile([128, T_TILE], I32, tag="ki", name="ki")
                    nc.vector.tensor_scalar(kq[:, :tt], mp[:, :tt], inv2pi, None,
                                            op0=ALU.mult)
                    nc.vector.tensor_copy(ki[:, :tt], kq[:, :tt])
                    nc.vector.tensor_copy(kq[:, :tt], ki[:, :tt])
                    red = work.tile([128, T_TILE], F32, tag="red", name="red")
                    nc.vector.cody_waite_cascade(red[:, :tt], mp[:, :tt], kq[:, :tt],
                                                 cw1, cw2, cw3)
                    sn = work.tile([128, T_TILE], F32, tag="sn", name="sn")
                    nc.scalar.activation(sn[:, :tt], red[:, :tt], ACT.Sin)
                    nc.vector.add_range_wrap(red[:, :tt], red[:, :tt], HPI, PI, TPI)
                    cs = work.tile([128, T_TILE], F32, tag="cs", name="cs")
                    nc.scalar.activation(cs[:, :tt], red[:, :tt], ACT.Sin)
                    nc.vector.tensor_mul(cs[:, :tt], cs[:, :tt], ENV[m][:, :tt])
                    nc.vector.tensor_mul(sn[:, :tt], sn[:, :tt], ENV[m][:, :tt])
                    tr = work.tile([128, T_TILE], F32, tag="mp", name="tr")
                    ti = work.tile([128, T_TILE], F32, tag="red", name="ti")
                    nc.vector.tensor_mul(tr[:, :tt], cs[:, :tt], Xre[m][:, :tt])
                    nc.vector.tensor_mul(ti[:, :tt], sn[:, :tt], Xim[m][:, :tt])
                    nc.vector.tensor_sub(tr[:, :tt], tr[:, :tt], ti[:, :tt])
                    nc.vector.tensor_mul(ti[:, :tt], cs[:, :tt], Xim[m][:, :tt])
                    nc.vector.tensor_mul(Xre[m][:, :tt], sn[:, :tt], Xre[m][:, :tt])
                    nc.vector.tensor_add(Xim[m][:, :tt], ti[:, :tt], Xre[m][:, :tt])
                    nc.vector.tensor_copy(Xre[m][:, :tt], tr[:, :tt])

            spec_mul(LNP, PEt, XPre, XPim)
            spec_mul(LNA, AEt, XAre, XAim)

            def irfft_oa(Xre, Xim, cr, wav_dr):
                ys = []
                for smc in range(4):
                    ps = pp.tile([128, T_TILE], F32, tag="ps", name="psi")
                    for kc in range(5):
                        nc.tensor.matmul(ps[:, :tt], ICt[:, kc, smc * 128:(smc + 1) * 128],
                                         Xre[kc][:, :tt], start=(kc == 0), stop=False)
                        nc.tensor.matmul(ps[:, :tt], ISt[:, kc, smc * 128:(smc + 1) * 128],
                                         Xim[kc][:, :tt], start=False, stop=(kc == 4))
                    yt = work.tile([128, T_TILE], F32, tag=f"y{smc}", name=f"yt{smc}")
                    nc.vector.tensor_copy(yt[:, :tt], ps[:, :tt])
                    ys.append(yt)
                for hh in range(2):
                    ncar = sm.tile([128, 1], F32, tag="ncar", name="ncar")
                    nc.vector.tensor_copy(ncar[:], ys[hh + 2][:, tt - 1:tt])
                    nc.vector.tensor_add(ys[hh][:, 1:tt], ys[hh][:, 1:tt],
                                         ys[hh + 2][:, 0:tt - 1])
                    nc.vector.tensor_add(ys[hh][:, 0:1], ys[hh][:, 0:1], cr[hh][:])
                    nc.vector.tensor_copy(cr[hh][:], ncar[:])
                    nc.sync.dma_start(out=wav_dr[hh * 128:(hh + 1) * 128, t0:t0 + tt],
                                      in_=ys[hh][:, :tt])

            irfft_oa(XPre, XPim, carry["p"], wp_dr)
            irfft_oa(XAre, XAim, carry["a"], wa_dr)

        # ================= pass 2: DC removal =================
        inv1024 = float(1.0 / 1024.0)
        for it in range(N_TILES):
            t0 = it * T_TILE
            tt = min(T_TILE, T - t0)
            lo = max(t0 - 2, 0)
            hi = min(t0 + tt + 2, T)
            w_ = hi - lo
            pos = lo - (t0 - 2)
            WPA = [work.tile([128, T_TILE + 4], F32, tag=f"WPA{h}", name=f"WPA{h}")
                   for h in range(2)]
            WAo = [work.tile([128, T_TILE + 4], F32, tag=f"WAo{h}", name=f"WAo{h}")
                   for h in range(2)]
            for h in range(2):
                nc.vector.memset(WPA[h][:], 0.0)
                nc.vector.memset(WAo[h][:], 0.0)
                nc.sync.dma_start(out=WPA[h][:, pos:pos + w_],
                                  in_=wp_dr[h * 128:(h + 1) * 128, lo:hi])
                nc.sync.dma_start(out=WAo[h][:, pos:pos + w_],
                                  in_=wa_dr[h * 128:(h + 1) * 128, lo:hi])
                nc.vector.tensor_add(WPA[h][:], WPA[h][:], WAo[h][:])

            for Wt_, outd in ((WPA, out0_d), (WAo, out1_d)):
                psc = pp.tile([1, T_TILE + 4], F32, tag="ps", name="psc")
                for h in range(2):
                    nc.tensor.matmul(psc[:, :], ONESCt[:, h, :], Wt_[h][:, :],
                                     start=(h == 0), stop=(h == 1))
                ct = sm.tile([1, T_TILE + 4], F32, tag="ct", name="ct")
                nc.vector.tensor_copy(ct[:], psc[:])
                et = sm.tile([1, T_TILE], F32, tag="et", name="et")
                nc.vector.tensor_add(et[:, :tt], ct[:, 0:tt], ct[:, 1:1 + tt])
                nc.vector.tensor_add(et[:, :tt], et[:, :tt], ct[:, 2:2 + tt])
                nc.vector.tensor_add(et[:, :tt], et[:, :tt], ct[:, 3:3 + tt])
                PLO = []
                for um in range(2):
                    psl = pp.tile([128, T_TILE + 4], F32, tag="ps", name="psl")
                    for h in range(2):
                        nc.tensor.matmul(psl[:, :], TRISt[:, h, um * 128:(um + 1) * 128],
                                         Wt_[h][:, :], start=(h == 0), stop=(h == 1))
                    pl = work.tile([128, T_TILE + 4], F32, tag=f"pl{um}", name=f"pl{um}")
                    nc.vector.tensor_copy(pl[:], psl[:])
                    PLO.append(pl)
                for um in range(2):
                    pse = pp.tile([128, T_TILE], F32, tag="ps", name="pse")
                    nc.tensor.matmul(pse[:, :tt], EXP1t[:, um * 128:(um + 1) * 128],
                                     et[:, :tt], start=True, stop=True)
                    ms = work.tile([128, T_TILE], F32, tag="mp", name="ms")
                    nc.vector.tensor_copy(ms[:, :tt], pse[:, :tt])
                    nc.vector.tensor_add(ms[:, :tt], ms[:, :tt], PLO[um][:, 4:4 + tt])
                    nc.vector.tensor_sub(ms[:, :tt], ms[:, :tt], PLO[um][:, 0:tt])
                    nc.vector.tensor_scalar(ms[:, :tt], ms[:, :tt], inv1024, None,
                                            op0=ALU.mult)
                    ow = work.tile([128, T_TILE], F32, tag="kq", name="ow")
                    nc.vector.tensor_sub(ow[:, :tt], Wt_[um][:, 2:2 + tt], ms[:, :tt])
                    for ch in range((tt + 127) // 128):
                        tw = min(128, tt - ch * 128)
                        pst = pp.tile([128, T_TILE], F32, tag="ps", name="pst")
                        nc.tensor.transpose(pst[:tw, :128], ow[:, ch * 128:ch * 128 + tw],
                                            IDNt[:, :])
                        ob = work.tile([128, 128], F32, tag="red", name="ob")
                        nc.vector.tensor_copy(ob[:tw, :], pst[:tw, :128])
                        ost = outd.ap[1][0]
                        nc.sync.dma_start(
                            out=bass.AP(tensor=outd.tensor,
                                        offset=outd.offset + ((t0 + ch * 128) * 256 + um * 128) * ost,
                                        ap=[[256 * ost, tw], [ost, 128]]),
                            in_=ob[:tw, :])

    nc.compile()
    return nc


_NC_CACHE = None
LAST_RESULTS = None


def kernel(f0_hz, spectral_env, aperiodicity, noise):
    global _NC_CACHE
    from concourse.bass_utils import run_bass_kernel_spmd

    if _NC_CACHE is None:
        _NC_CACHE = _build_program()
    nc = _NC_CACHE

    c = _CONST
    in_maps = []
    for b in range(B):
        g = np.asarray(f0_hz[b], dtype=f32)
        gpad = np.empty((1, 4002), dtype=f32)
        gpad[0, 0] = g[0]
        gpad[0, 1:4001] = g
        gpad[0, 4001] = g[-1]
        bnd = np.empty((1, 256), dtype=f32)
        bnd[0, :128] = g[0]
        bnd[0, 128:] = g[-1]
        in_maps.append({
            "g": gpad, "BNDc": bnd,
            "env": np.ascontiguousarray(spectral_env[b], dtype=f32),
            "apr": np.ascontiguousarray(aperiodicity[b], dtype=f32),
            "noi": np.ascontiguousarray(noise[b], dtype=f32).reshape(1, N),
            "W1c": c["W1"], "W2c": c["W2"], "MAc": c["MA"],
            "RCc": c["RC"], "RSc": c["RS"], "ICc": c["IC"], "ISc": c["IS"],
            "TRISc": c["TRIS"], "ONESCc": c["ONESC"], "EXP1c": c["EXP1"],
            "IDNc": c["IDN"],
        })

    import os
    trace = bool(int(os.environ.get("KERNEL_TRACE", "0")))
    res = run_bass_kernel_spmd(nc, in_maps, core_ids=list(range(B)), trace=trace)
    global LAST_RESULTS
    LAST_RESULTS = res
    out0 = np.stack([res.results[b]["out0"].reshape(N) for b in range(B)])
    out1 = np.stack([res.results[b]["out1"].reshape(N) for b in range(B)])
    return out0.astype(f32), out1.astype(f32)
